# revision 6
# baseline (speedup 1.0000x reference)
"""Trainium2 Bass kernel for nn_CLoss (topk_masking), 8-core SPMD.

Semantics (see reference):
  t_logit[i] = output[i, target[i]]
  margin[i]  = t_logit[i] - max_{k != target[i]} output[i, k]
  lse[i]     = logsumexp(output[i, :])
  l[i]       = max(0, margin>0 ? 1-margin : 1 - t_logit + lse)
  sort margins ascending; v[index[i]] = 1 iff cumsum(sorted)[i] <= thr + 1 - i
  c1 = v . l ;  c2 = B - sum(v) + #(margin<0) ;  out = min(c1, c2)

Strategy (data-parallel over batch):
  - Each core streams its [512, 50257] row shard once in [128, 8192]
    chunks (4 MB DMAs for ~90%+ DMA efficiency): DVE max-reduce + ACT
    Exp+accum run under the DMA stream.
  - All small per-tile work is deferred/batched so the tile scheduler
    cannot interleave serial chains (Ln table swaps, l-epilogue) into
    the stream -- that was measured to stall the sync engine's DMA
    issue at tile boundaries.
  - t_logit for all 4 tiles gathered upfront via indirect DMA.
  - Per-tile margin AllGather + stride-0 broadcast are issued on gpsimd
    MID-STREAM (tile t's collective overlaps tile t+1's streaming), so
    only the last tile's AllGather latency is exposed at stream end.
  - Sort-free selection, two full-width passes on different engines:
      ACT:  A_j = sum_k relu(m_j - m_k)
      DVE:  n_j = #{m_k < m_j}
      keep: v_j = [(n_j+1)(m_j+1) - A_j <= thr + 2]
  - Per-core partials (v.l, sum v, #neg) via ones-matmul, tiny
    AllGather + local reduce; every core computes min(c1, c2).
"""

import numpy as np

import concourse.bass as bass
import concourse.bacc as bacc
import concourse.tile as tile
from concourse import mybir
from concourse.bass_utils import run_bass_kernel_spmd

B_FULL, C_FULL, N_CORES = 4096, 50257, 8
P = 128
CHUNK = 8192

F32 = mybir.dt.float32
I32 = mybir.dt.int32
ALU = mybir.AluOpType
ACTF = mybir.ActivationFunctionType
AX = mybir.AxisListType


def _chunks(c, f):
    out, off = [], 0
    while off < c:
        out.append((off, min(f, c - off)))
        off += f if off + f <= c else c - off
    return out


def build_nc(threshold, b=B_FULL, c=C_FULL, n_cores=N_CORES, chunk=CHUNK):
    thr = float(threshold)
    R = b // n_cores
    T = R // P
    G = P * n_cores  # margins per tile-gather (1024)
    assert R % P == 0 and b % n_cores == 0

    nc = bacc.Bacc("TRN2", target_bir_lowering=False, debug=False,
                   num_devices=n_cores)
    x = nc.dram_tensor("x", [R, c], F32, kind="ExternalInput")
    tgt = nc.dram_tensor("tgtflat", [P, T], I32, kind="ExternalInput")
    out_ext = nc.dram_tensor("out", [1, 1], F32, kind="ExternalOutput")
    x_flat = x.ap().rearrange("a (b one) -> (a b) one", one=1)

    chs = _chunks(c, chunk)
    nch = len(chs)

    with tile.TileContext(nc) as tc:
        with tc.tile_pool(name="io", bufs=3) as io_pool, \
             tc.tile_pool(name="scr", bufs=2) as scr_pool, \
             tc.tile_pool(name="stats", bufs=2) as stats_pool, \
             tc.tile_pool(name="small", bufs=1) as small, \
             tc.tile_pool(name="psum", bufs=1, space="PSUM") as psum_pool, \
             tc.tile_pool(name="dram", bufs=1, space="DRAM") as dram:

            # margins exchanged in two groups: tiles 0-2 (AllGather fired at
            # ~75% of the stream, overlapping the last tile's streaming) and
            # tile 3 (fired at stream end, overlapped with selection below)
            mg_loc_a = dram.tile([3 * P], F32, tag="mg_loc_a")
            mg_all_a = dram.tile([3 * P * n_cores], F32, tag="mg_all_a")
            mg_loc_b = dram.tile([P], F32, tag="mg_loc_b")
            mg_all_b = dram.tile([P * n_cores], F32, tag="mg_all_b")
            W_A = 3 * P * n_cores  # 3072 columns from tiles 0-2
            W_B = P * n_cores      # 1024 columns from tile 3
            part_local = dram.tile([8], F32, tag="part_local")
            part_gath = dram.tile([8 * n_cores], F32, tag="part_gath")

            # upfront: target indices + t_logit gather for all tiles
            idx = small.tile([P, T], I32, tag="idx")
            nc.sync.dma_start(out=idx[:], in_=tgt.ap()[:, :])
            tl4 = small.tile([P, T], F32, tag="tl4")
            for t in range(T):
                nc.gpsimd.indirect_dma_start(
                    out=tl4[:, t:t + 1], out_offset=None, in_=x_flat,
                    in_offset=bass.IndirectOffsetOnAxis(ap=idx[:, t:t + 1],
                                                        axis=0))

            margin4 = small.tile([P, T], F32, tag="margin4")
            S4 = small.tile([P, T], F32, tag="S4")
            mb = small.tile([P, b], F32, tag="mb")

            for t in range(T):
                maxcols = stats_pool.tile([P, nch], F32, tag="maxcols")
                sumcols = stats_pool.tile([P, nch], F32, tag="sumcols")
                for i, (off, f) in enumerate(chs):
                    it = io_pool.tile([P, chunk], F32, tag="in")
                    nc.sync.dma_start(out=it[:, :f],
                                      in_=x.ap()[t * P:(t + 1) * P, off:off + f])
                    nc.vector.tensor_reduce(out=maxcols[:, i:i + 1], in_=it[:, :f],
                                            axis=AX.X, op=ALU.max)
                    es = scr_pool.tile([P, chunk], F32, tag="es")
                    nc.scalar.activation(out=es[:, :f], in_=it[:, :f],
                                         func=ACTF.Exp,
                                         accum_out=sumcols[:, i:i + 1])

                rowmax = small.tile([P, 1], F32, tag=f"rowmax{t}")
                nc.vector.tensor_reduce(out=rowmax[:], in_=maxcols[:], axis=AX.X,
                                        op=ALU.max)
                nc.vector.tensor_reduce(out=S4[:, t:t + 1], in_=sumcols[:],
                                        axis=AX.X, op=ALU.add)
                nc.vector.tensor_tensor(out=margin4[:, t:t + 1],
                                        in0=tl4[:, t:t + 1], in1=rowmax[:],
                                        op=ALU.subtract)
                # margin store + AllGather + partition-broadcast, all on
                # gpsimd: overlaps the remaining stream; sync/ACT/DVE never
                # wait on these mid-stream.
                if t < 3:
                    nc.gpsimd.dma_start(out=mg_loc_a[t * P:(t + 1) * P],
                                        in_=margin4[:, t:t + 1])
                else:
                    nc.gpsimd.dma_start(out=mg_loc_b[:],
                                        in_=margin4[:, t:t + 1])
                if t == 2:
                    nc.gpsimd.collective_compute(
                        "AllGather", ALU.bypass,
                        ins=[mg_loc_a[:].opt()], outs=[mg_all_a[:].opt()],
                        replica_groups=[list(range(n_cores))])
                    bc_a = bass.AP(mg_all_a[:].tensor, mg_all_a[:].offset,
                                   [[0, P], [1, W_A]])
                    nc.gpsimd.dma_start(out=mb[:, 0:W_A], in_=bc_a)
                if t == 3:
                    nc.gpsimd.collective_compute(
                        "AllGather", ALU.bypass,
                        ins=[mg_loc_b[:].opt()], outs=[mg_all_b[:].opt()],
                        replica_groups=[list(range(n_cores))])
                    bc_b = bass.AP(mg_all_b[:].tensor, mg_all_b[:].offset,
                                   [[0, P], [1, W_B]])
                    nc.gpsimd.dma_start(out=mb[:, W_A:W_A + W_B], in_=bc_b)

            # ---- tail (everything below depends on all 4 tiles) ----
            # l = max(0, a + gt*(bb-a)), a = 1 - tl + lse, bb = 1 - margin
            lse4 = small.tile([P, T], F32, tag="lse4")
            nc.scalar.activation(out=lse4[:], in_=S4[:], func=ACTF.Ln)
            a1 = small.tile([P, T], F32, tag="a1")
            nc.vector.tensor_tensor(out=a1[:], in0=lse4[:], in1=tl4[:],
                                    op=ALU.subtract)
            a4 = small.tile([P, T], F32, tag="a4")
            nc.vector.tensor_scalar(out=a4[:], in0=a1[:], scalar1=1.0,
                                    scalar2=None, op0=ALU.add)
            bb4 = small.tile([P, T], F32, tag="bb4")
            nc.vector.tensor_scalar(out=bb4[:], in0=margin4[:], scalar1=-1.0,
                                    scalar2=1.0, op0=ALU.mult, op1=ALU.add)
            gt4 = small.tile([P, T], F32, tag="gt4")
            nc.vector.tensor_scalar(out=gt4[:], in0=margin4[:], scalar1=0.0,
                                    scalar2=None, op0=ALU.is_gt)
            d1 = small.tile([P, T], F32, tag="d1")
            nc.vector.tensor_tensor(out=d1[:], in0=bb4[:], in1=a4[:],
                                    op=ALU.subtract)
            d2 = small.tile([P, T], F32, tag="d2")
            nc.vector.tensor_tensor(out=d2[:], in0=gt4[:], in1=d1[:],
                                    op=ALU.mult)
            lpre = small.tile([P, T], F32, tag="lpre")
            nc.vector.tensor_tensor(out=lpre[:], in0=a4[:], in1=d2[:],
                                    op=ALU.add)
            l4 = small.tile([P, T], F32, tag="l4")
            nc.vector.tensor_scalar(out=l4[:], in0=lpre[:], scalar1=0.0,
                                    scalar2=None, op0=ALU.max)

            # concurrent selection passes: ACT computes A, DVE computes n.
            # Split over the two margin groups so the wide group-a pass runs
            # while group-b's AllGather is still in flight.
            A4a = small.tile([P, T], F32, tag="A4a")
            n4a = small.tile([P, T], F32, tag="n4a")
            A4b = small.tile([P, T], F32, tag="A4b")
            n4b = small.tile([P, T], F32, tag="n4b")
            for t in range(T):
                selA = scr_pool.tile([P, chunk], F32, tag="es")
                nc.scalar.activation(out=selA[:, :W_A], in_=mb[:, 0:W_A],
                                     func=ACTF.Relu,
                                     scale=-1.0, bias=margin4[:, t:t + 1],
                                     accum_out=A4a[:, t:t + 1])
                selL = scr_pool.tile([P, chunk], F32, tag="es")
                nc.vector.tensor_scalar(out=selL[:, :W_A], in0=mb[:, 0:W_A],
                                        scalar1=margin4[:, t:t + 1],
                                        scalar2=None,
                                        op0=ALU.is_lt, op1=ALU.add,
                                        accum_out=n4a[:, t:t + 1])
            for t in range(T):
                selA = scr_pool.tile([P, chunk], F32, tag="es")
                nc.scalar.activation(out=selA[:, :W_B],
                                     in_=mb[:, W_A:W_A + W_B], func=ACTF.Relu,
                                     scale=-1.0, bias=margin4[:, t:t + 1],
                                     accum_out=A4b[:, t:t + 1])
                selL = scr_pool.tile([P, chunk], F32, tag="es")
                nc.vector.tensor_scalar(out=selL[:, :W_B],
                                        in0=mb[:, W_A:W_A + W_B],
                                        scalar1=margin4[:, t:t + 1],
                                        scalar2=None,
                                        op0=ALU.is_lt, op1=ALU.add,
                                        accum_out=n4b[:, t:t + 1])
            A4 = small.tile([P, T], F32, tag="A4")
            n4 = small.tile([P, T], F32, tag="n4")
            nc.vector.tensor_tensor(out=A4[:], in0=A4a[:], in1=A4b[:],
                                    op=ALU.add)
            nc.vector.tensor_tensor(out=n4[:], in0=n4a[:], in1=n4b[:],
                                    op=ALU.add)

            # keep test: v = [(n+1)(m+1) - A <= thr + 2]
            e1 = small.tile([P, T], F32, tag="e1")
            nc.vector.tensor_scalar(out=e1[:], in0=n4[:], scalar1=1.0,
                                    scalar2=None, op0=ALU.add)
            e2 = small.tile([P, T], F32, tag="e2")
            nc.vector.tensor_scalar(out=e2[:], in0=margin4[:], scalar1=1.0,
                                    scalar2=None, op0=ALU.add)
            e3 = small.tile([P, T], F32, tag="e3")
            nc.vector.tensor_tensor(out=e3[:], in0=e1[:], in1=e2[:],
                                    op=ALU.mult)
            dd = small.tile([P, T], F32, tag="dd")
            nc.vector.tensor_tensor(out=dd[:], in0=e3[:], in1=A4[:],
                                    op=ALU.subtract)
            v4 = small.tile([P, T], F32, tag="v4")
            nc.vector.tensor_scalar(out=v4[:], in0=dd[:],
                                    scalar1=thr + 2.0, scalar2=None,
                                    op0=ALU.is_le)
            neg4 = small.tile([P, T], F32, tag="neg4")
            nc.vector.tensor_scalar(out=neg4[:], in0=margin4[:], scalar1=0.0,
                                    scalar2=None, op0=ALU.is_lt)
            st12 = small.tile([P, 3 * T], F32, tag="st12")
            nc.vector.tensor_tensor(out=st12[:, 0:T], in0=v4[:], in1=l4[:],
                                    op=ALU.mult)
            nc.vector.tensor_copy(out=st12[:, T:2 * T], in_=v4[:])
            nc.vector.tensor_copy(out=st12[:, 2 * T:3 * T], in_=neg4[:])

            ones = small.tile([P, 1], F32, tag="ones")
            nc.vector.memset(ones[:], 1.0)
            acc = psum_pool.tile([1, 3 * T], F32)
            nc.tensor.matmul(out=acc[:], lhsT=ones[:], rhs=st12[:],
                             start=True, stop=True)
            # reduce the per-tile groups -> [1,3] partials
            acc_sb = small.tile([1, 3 * T], F32, tag="acc_sb")
            nc.vector.tensor_copy(out=acc_sb[:], in_=acc[:])
            accs = small.tile([1, 8], F32, tag="accs")
            nc.vector.memset(accs[:], 0.0)
            nc.vector.tensor_reduce(
                out=accs[:, 0:3],
                in_=acc_sb[:].rearrange("p (g tt) -> p g tt", tt=T),
                axis=AX.X, op=ALU.add)
            nc.sync.dma_start(out=part_local[:], in_=accs[:])
            # tiny partial exchange: AllGather floor beats AllReduce floor
            nc.gpsimd.collective_compute(
                "AllGather", ALU.bypass,
                ins=[part_local[:].opt()], outs=[part_gath[:].opt()],
                replica_groups=[list(range(n_cores))])
            # value-major transposed load, then reduce over cores
            tot88 = small.tile([1, 8 * n_cores], F32, tag="tot88")
            gsrc = bass.AP(part_gath[:].tensor, part_gath[:].offset,
                           [[0, 1], [1, 8], [8, n_cores]])
            nc.sync.dma_start(out=tot88[:], in_=gsrc)
            tot = small.tile([1, 8], F32, tag="tot")
            nc.vector.tensor_reduce(
                out=tot[:],
                in_=tot88[:].rearrange("p (vv cc) -> p vv cc", cc=n_cores),
                axis=AX.X, op=ALU.add)
            c2a = small.tile([1, 1], F32, tag="c2a")
            nc.vector.tensor_scalar(out=c2a[:], in0=tot[:, 1:2], scalar1=-1.0,
                                    scalar2=float(b), op0=ALU.mult, op1=ALU.add)
            c2 = small.tile([1, 1], F32, tag="c2")
            nc.vector.tensor_tensor(out=c2[:], in0=c2a[:], in1=tot[:, 2:3],
                                    op=ALU.add)
            res = small.tile([1, 1], F32, tag="res")
            nc.vector.tensor_tensor(out=res[:], in0=tot[:, 0:1], in1=c2[:],
                                    op=ALU.min)
            nc.sync.dma_start(out=out_ext.ap()[:], in_=res[:])

    nc.compile()
    return nc


def make_in_maps(output, target, b, c, n_cores, chunk=None):
    output = np.ascontiguousarray(np.asarray(output, dtype=np.float32))
    target = np.asarray(target).astype(np.int64)
    R = b // n_cores
    T = R // P
    rows = np.arange(R, dtype=np.int64)
    in_maps = []
    for cc in range(n_cores):
        tsh = target[cc * R:(cc + 1) * R]
        flat = (rows * c + tsh).astype(np.int32)          # [R]
        tile4 = np.ascontiguousarray(flat.reshape(T, P).T)  # [P, T]
        in_maps.append({
            "x": output[cc * R:(cc + 1) * R],
            "tgtflat": tile4,
        })
    return in_maps


_NC_CACHE = {}


def kernel(output, target, threshold):
    """Full inputs in, full (scalar) output out; shards + runs on 8 cores."""
    thr = float(np.asarray(threshold))
    if thr not in _NC_CACHE:
        _NC_CACHE[thr] = build_nc(thr)
    nc = _NC_CACHE[thr]
    in_maps = make_in_maps(output, target, B_FULL, C_FULL, N_CORES)
    res = run_bass_kernel_spmd(nc, in_maps, core_ids=list(range(N_CORES)))
    val = np.float32(res.results[0]["out"][0, 0])
    return np.asarray(val, dtype=np.float32)
